# revision 16
# baseline (speedup 1.0000x reference)
"""Trainium2 Bass kernel for Chebyshev (L-inf) "convolution".

Math (see reference):
  out[b,co,h,w] = max_n |weights[co,n] - x_pad[b, c(co,n), h+di(co,n), w+dj(co,n)]| + bias[co]
  where conn_idx[co,n] = c*9 + di*3 + dj and x_pad is replicate-padded by 1.

Strategy (8 NeuronCores, batch-sharded: 4 images per core):
  1. Per image: load x into SBUF, build a replicate-padded bf16 plane set
     [C=64, 66*66] via an on-engine strided cast-copy, store contiguously to
     DRAM scratch xpad_b.
  2. Per (image, tap): one indirect DMA; output partition co reads a
     contiguous 4222-element span of xpad_b at element offset
     c*4356 + di*66 + dj.  The shifted 64x64 window sits at row-stride 66.
  3. Per tap: d_n = G_n + (-w_n) written into slot n of a contiguous
     D[co, 4, 4096] tile -- a single-ALU tensor_scalar (VectorE ~3 elem/cyc;
     two taps go to ScalarE as Identity-with-bias to balance engines).
  4. ONE VectorE tensor_reduce(max, apply_absolute_value=True) over a
     transposed AP view D[co, s, n] (n innermost) collapses abs AND the
     whole max tree: mf[co, s] = max_n |d_n[co, s]|.
  5. Output stored bf16 (halves store traffic); bias-add + f32 upcast happen
     on the host (exact f32 math, fused with the unshard pass).

DMA-byte-bound: ~28 MB/core over 16 DMA engines => ~76+ us floor; engine
work (~35-45 us per engine) hides underneath.
"""

import numpy as np

B, CIN, H, W = 32, 64, 64, 64
COUT, NCONN = 128, 4
KH, KW = 3, 3
NCORES = 8
BL = B // NCORES            # 4 images per core
PH, PW = H + 2, W + 2       # 66 x 66 replicate-padded planes
PLANE = PH * PW             # 4356
S = H * W                   # 4096
SPAN = (H - 1) * PW + W     # 4222: span holding one shifted 64x64 window
GPAD = SPAN + 2             # 4224 (even) SBUF tile width

_CACHE = {}


def _build_program():
    import concourse.bass as bass
    import concourse.bacc as bacc
    import concourse.mybir as mybir
    from concourse.tile import TileContext, add_dep_helper

    f32 = mybir.dt.float32
    bf16 = mybir.dt.bfloat16
    i32 = mybir.dt.int32
    Alu = mybir.AluOpType
    Act = mybir.ActivationFunctionType

    nc = bacc.Bacc("TRN2", target_bir_lowering=False, debug=False, num_swdge_queues=2)

    x_ext = nc.dram_tensor("x", (BL, CIN, H, W), bf16, kind="ExternalInput").ap()
    wneg_ext = nc.dram_tensor("wneg", (COUT, NCONN), f32, kind="ExternalInput").ap()
    gidx_ext = nc.dram_tensor(
        "gidx", (COUT, BL * NCONN * 8), i32, kind="ExternalInput"
    ).ap()
    out_ext = [
        nc.dram_tensor(f"out{b}", (COUT, H, W), bf16, kind="ExternalOutput").ap()
        for b in range(BL)
    ]
    xpads = [
        nc.dram_tensor(f"xpad{b}", (CIN * PLANE, 1), bf16) for b in range(BL)
    ]

    with TileContext(nc, pool_alloc_mode="queue") as tc:
        with (
            tc.tile_pool(name="const", bufs=1) as cpool,
            tc.tile_pool(name="xs", bufs=2) as xspool,
            tc.tile_pool(name="xp", bufs=2) as xppool,
            tc.tile_pool(name="g", bufs=8) as gpool,
            tc.tile_pool(name="t", bufs=6) as dpool,
            tc.tile_pool(name="m", bufs=4) as mpool,
            tc.tile_pool(name="o", bufs=2) as opool,
        ):
            wneg_sb = cpool.tile([COUT, NCONN], f32)
            nc.sync.dma_start(out=wneg_sb[:], in_=wneg_ext)
            gidx_sb = cpool.tile([COUT, BL * NCONN * 8], i32)
            nc.sync.dma_start(out=gidx_sb[:], in_=gidx_ext)

            for b in range(BL):
                # --- padded bf16 planes for image b (VectorE cast+pad) ---
                XSB = xspool.tile([CIN, S], bf16, tag="xsb")
                nc.sync.dma_start(
                    out=XSB[:], in_=x_ext[b].rearrange("c h w -> c (h w)")
                )
                XP = xppool.tile([CIN, PLANE], bf16, tag="xp")
                XPv = XP[:].rearrange("c (h w) -> c h w", h=PH, w=PW)
                nc.vector.tensor_copy(
                    out=XPv[:, 1 : H + 1, 1 : W + 1],
                    in_=XSB[:].rearrange("c (h w) -> c h w", h=H, w=W),
                )
                nc.vector.tensor_copy(
                    out=XPv[:, 1 : H + 1, 0:1], in_=XPv[:, 1 : H + 1, 1:2]
                )
                nc.vector.tensor_copy(
                    out=XPv[:, 1 : H + 1, PW - 1 : PW],
                    in_=XPv[:, 1 : H + 1, PW - 2 : PW - 1],
                )
                nc.vector.tensor_copy(out=XPv[:, 0:1, :], in_=XPv[:, 1:2, :])
                nc.vector.tensor_copy(
                    out=XPv[:, PH - 1 : PH, :], in_=XPv[:, PH - 2 : PH - 1, :]
                )
                store = nc.sync.dma_start(
                    out=xpads[b].ap().rearrange(
                        "(c p) one -> c (p one)", c=CIN, p=PLANE
                    ),
                    in_=XP[:],
                )

                # --- per tap: indirect span gather + |G - w| ---
                gvs = [None] * NCONN
                for n in (3, 0, 1, 2):
                    k = b * NCONN + n
                    gt = gpool.tile([COUT, GPAD], bf16, tag="g")
                    gather = nc.gpsimd.indirect_dma_start(
                        out=gt[:, 0:SPAN],
                        out_offset=None,
                        in_=xpads[b].ap(),
                        in_offset=bass.IndirectOffsetOnAxis(
                            ap=gidx_sb[:, k * 8 : k * 8 + 1], axis=0
                        ),
                    )
                    if (b * NCONN + n) % 2 == 1:
                        gather.ins.queue = "qPoolDynamic1"
                    add_dep_helper(
                        gather.ins, store.ins, reason="gather reads xpad[b]"
                    )
                    gvs[n] = (
                        gt[:].rearrange("p (h w) -> p h w", h=H, w=PW)[:, :, 0:W]
                    )

                # tap 3 (gathered first) on VectorE via two cheap
                # tensor_scalars: d3 = G + (-w), e3 = w - G, |G-w| = max
                d3 = dpool.tile([COUT, S], bf16, tag="t")
                nc.vector.tensor_scalar(
                    out=d3[:].rearrange("p (h w) -> p h w", h=H, w=W),
                    in0=gvs[3], scalar1=wneg_sb[:, 3:4], scalar2=None,
                    op0=Alu.add,
                )
                e3 = dpool.tile([COUT, S], bf16, tag="t")
                nc.vector.tensor_scalar(
                    out=e3[:].rearrange("p (h w) -> p h w", h=H, w=W),
                    in0=gvs[3], scalar1=-1.0, scalar2=wneg_sb[:, 3:4],
                    op0=Alu.mult, op1=Alu.subtract,
                )
                m3 = mpool.tile([COUT, S], bf16, tag="m")
                nc.vector.tensor_tensor(
                    out=m3[:], in0=d3[:], in1=e3[:], op=Alu.max
                )
                # taps 0-2: |G - w| on ScalarE (Abs activation, bias = -w)
                ts_ = []
                for n in range(3):
                    t = dpool.tile([COUT, S], bf16, tag="t")
                    nc.scalar.activation(
                        out=t[:].rearrange("p (h w) -> p h w", h=H, w=W),
                        in_=gvs[n], func=Act.Abs,
                        bias=wneg_sb[:, n : n + 1], scale=1.0,
                    )
                    ts_.append(t)
                # merge tree on VectorE; tap-2 (last gathered) is the tail
                m01 = mpool.tile([COUT, S], bf16, tag="m")
                nc.vector.tensor_tensor(
                    out=m01[:], in0=ts_[0][:], in1=ts_[1][:], op=Alu.max
                )
                m013 = mpool.tile([COUT, S], bf16, tag="m")
                nc.vector.tensor_tensor(
                    out=m013[:], in0=m01[:], in1=m3[:], op=Alu.max
                )
                # final max + store in halves (overlap compute with store)
                outv = out_ext[b].rearrange("c h w -> c (h w)")
                for hh in range(2):
                    sl = slice(hh * (S // 2), (hh + 1) * (S // 2))
                    mf = opool.tile([COUT, S // 2], bf16, tag="o")
                    nc.vector.tensor_tensor(
                        out=mf[:], in0=m013[:, sl], in1=ts_[2][:, sl], op=Alu.max
                    )
                    # out-store on the Scalar-engine HWDGE queue
                    nc.scalar.dma_start(out=outv[:, sl], in_=mf[:])
    nc.compile()
    return nc


def _host_inputs(x, weights, bias, conn_idx):
    """Per-core input maps (host-side prep: shard x, derive -w / gather
    row-indices from the tiny weight/index tensors)."""
    ci = np.asarray(conn_idx).astype(np.int64)          # [COUT, NCONN]
    c = ci // (KH * KW)
    rem = ci % (KH * KW)
    di = rem // KW
    dj = rem % KW
    # element offset into xpad_b [64, 66, 66]: c*4356 + di*66 + dj
    offs = (c * PLANE + di * PW + dj).astype(np.int32)          # [COUT, NCONN]
    gidx = np.zeros((COUT, BL * NCONN * 8), dtype=np.int32)
    for bb in range(BL):
        for n in range(NCONN):
            k = bb * NCONN + n
            gidx[:, k * 8] = offs[:, n]
    wneg = (-np.asarray(weights)).astype(np.float32)
    import ml_dtypes
    x = np.ascontiguousarray(np.asarray(x)).astype(ml_dtypes.bfloat16)
    in_maps = []
    for kcore in range(NCORES):
        in_maps.append(
            {
                "x": x[kcore * BL : (kcore + 1) * BL],
                "wneg": wneg,
                "gidx": gidx,
            }
        )
    return in_maps


def kernel(x, weights, bias, conn_idx):
    from concourse.bass_utils import run_bass_kernel_spmd

    if "nc" not in _CACHE:
        _CACHE["nc"] = _build_program()
    nc = _CACHE["nc"]
    in_maps = _host_inputs(x, weights, bias, conn_idx)
    res = run_bass_kernel_spmd(nc, in_maps, list(range(NCORES)))
    bias_f = np.asarray(bias, dtype=np.float32).reshape(1, COUT, 1, 1)
    outs = []
    for k in range(NCORES):
        a = np.stack(
            [
                np.asarray(res.results[k][f"out{b}"]).astype(np.float32)
                for b in range(BL)
            ]
        )
        outs.append(a + bias_f)    # exact f32 bias add on host
    return np.concatenate(outs, axis=0)


if __name__ == "__main__":
    nc = _build_program()
    print("program built OK")


# revision 17
# speedup vs baseline: 1.1206x; 1.1206x over previous
"""Trainium2 Bass kernel for Chebyshev (L-inf) "convolution".

Math (see reference):
  out[b,co,h,w] = max_n |weights[co,n] - x_pad[b, c(co,n), h+di(co,n), w+dj(co,n)]| + bias[co]
  where conn_idx[co,n] = c*9 + di*3 + dj and x_pad is replicate-padded by 1.

Strategy (8 NeuronCores, batch-sharded: 4 images per core):
  1. Per image: load x into SBUF, build a replicate-padded bf16 plane set
     [C=64, 66*66] via an on-engine strided cast-copy, store contiguously to
     DRAM scratch xpad_b.
  2. Per (image, tap): one indirect DMA; output partition co reads a
     contiguous 4222-element span of xpad_b at element offset
     c*4356 + di*66 + dj.  The shifted 64x64 window sits at row-stride 66.
  3. Per tap: d_n = G_n + (-w_n) written into slot n of a contiguous
     D[co, 4, 4096] tile -- a single-ALU tensor_scalar (VectorE ~3 elem/cyc;
     two taps go to ScalarE as Identity-with-bias to balance engines).
  4. ONE VectorE tensor_reduce(max, apply_absolute_value=True) over a
     transposed AP view D[co, s, n] (n innermost) collapses abs AND the
     whole max tree: mf[co, s] = max_n |d_n[co, s]|.
  5. Output stored bf16 (halves store traffic); bias-add + f32 upcast happen
     on the host (exact f32 math, fused with the unshard pass).

DMA-byte-bound: ~28 MB/core over 16 DMA engines => ~76+ us floor; engine
work (~35-45 us per engine) hides underneath.
"""

import numpy as np

B, CIN, H, W = 32, 64, 64, 64
COUT, NCONN = 128, 4
KH, KW = 3, 3
NCORES = 8
BL = B // NCORES            # 4 images per core
PH, PW = H + 2, W + 2       # 66 x 66 replicate-padded planes
PLANE = PH * PW             # 4356
S = H * W                   # 4096
SPAN = (H - 1) * PW + W     # 4222: span holding one shifted 64x64 window
GPAD = SPAN + 2             # 4224 (even) SBUF tile width

_CACHE = {}


def _build_program():
    import concourse.bass as bass
    import concourse.bacc as bacc
    import concourse.mybir as mybir
    from concourse.tile import TileContext, add_dep_helper

    f32 = mybir.dt.float32
    bf16 = mybir.dt.bfloat16
    i32 = mybir.dt.int32
    Alu = mybir.AluOpType
    Act = mybir.ActivationFunctionType

    nc = bacc.Bacc("TRN2", target_bir_lowering=False, debug=False, num_swdge_queues=2)

    x_ext = nc.dram_tensor("x", (BL, CIN, H, W), bf16, kind="ExternalInput").ap()
    wneg_ext = nc.dram_tensor("wneg", (COUT, NCONN), f32, kind="ExternalInput").ap()
    gidx_ext = nc.dram_tensor(
        "gidx", (COUT, BL * NCONN * 8), i32, kind="ExternalInput"
    ).ap()
    out_ext = [
        nc.dram_tensor(f"out{b}", (COUT, H, W), bf16, kind="ExternalOutput").ap()
        for b in range(BL)
    ]
    xpads = [
        nc.dram_tensor(f"xpad{b}", (CIN * PLANE, 1), bf16) for b in range(BL)
    ]

    with TileContext(nc, pool_alloc_mode="queue") as tc:
        with (
            tc.tile_pool(name="const", bufs=1) as cpool,
            tc.tile_pool(name="xs", bufs=2) as xspool,
            tc.tile_pool(name="xp", bufs=2) as xppool,
            tc.tile_pool(name="g", bufs=5) as gpool,
            tc.tile_pool(name="t", bufs=6) as dpool,
            tc.tile_pool(name="m", bufs=4) as mpool,
            tc.tile_pool(name="o", bufs=2) as opool,
        ):
            wneg_sb = cpool.tile([COUT, NCONN], f32)
            nc.sync.dma_start(out=wneg_sb[:], in_=wneg_ext)
            gidx_sb = cpool.tile([COUT, BL * NCONN * 8], i32)
            nc.sync.dma_start(out=gidx_sb[:], in_=gidx_ext)

            for b in range(BL):
                # --- padded bf16 planes for image b (VectorE cast+pad) ---
                XSB = xspool.tile([CIN, S], bf16, tag="xsb")
                nc.sync.dma_start(
                    out=XSB[:], in_=x_ext[b].rearrange("c h w -> c (h w)")
                )
                XP = xppool.tile([CIN, PLANE], bf16, tag="xp")
                XPv = XP[:].rearrange("c (h w) -> c h w", h=PH, w=PW)
                nc.vector.tensor_copy(
                    out=XPv[:, 1 : H + 1, 1 : W + 1],
                    in_=XSB[:].rearrange("c (h w) -> c h w", h=H, w=W),
                )
                nc.vector.tensor_copy(
                    out=XPv[:, 1 : H + 1, 0:1], in_=XPv[:, 1 : H + 1, 1:2]
                )
                nc.vector.tensor_copy(
                    out=XPv[:, 1 : H + 1, PW - 1 : PW],
                    in_=XPv[:, 1 : H + 1, PW - 2 : PW - 1],
                )
                nc.vector.tensor_copy(out=XPv[:, 0:1, :], in_=XPv[:, 1:2, :])
                nc.vector.tensor_copy(
                    out=XPv[:, PH - 1 : PH, :], in_=XPv[:, PH - 2 : PH - 1, :]
                )
                store = nc.sync.dma_start(
                    out=xpads[b].ap().rearrange(
                        "(c p) one -> c (p one)", c=CIN, p=PLANE
                    ),
                    in_=XP[:],
                )

                # --- per tap: indirect span gather + |G - w| ---
                gvs = [None] * NCONN
                for n in (3, 0, 1, 2):
                    k = b * NCONN + n
                    gt = gpool.tile([COUT, GPAD], bf16, tag="g")
                    gather = nc.gpsimd.indirect_dma_start(
                        out=gt[:, 0:SPAN],
                        out_offset=None,
                        in_=xpads[b].ap(),
                        in_offset=bass.IndirectOffsetOnAxis(
                            ap=gidx_sb[:, k * 8 : k * 8 + 1], axis=0
                        ),
                    )
                    if (b * NCONN + n) % 2 == 1:
                        gather.ins.queue = "qPoolDynamic1"
                    add_dep_helper(
                        gather.ins, store.ins, reason="gather reads xpad[b]"
                    )
                    gvs[n] = (
                        gt[:].rearrange("p (h w) -> p h w", h=H, w=PW)[:, :, 0:W]
                    )

                # tap 3 (gathered first) on VectorE via two cheap
                # tensor_scalars: d3 = G + (-w), e3 = w - G, |G-w| = max
                d3 = dpool.tile([COUT, S], bf16, tag="t")
                nc.vector.tensor_scalar(
                    out=d3[:].rearrange("p (h w) -> p h w", h=H, w=W),
                    in0=gvs[3], scalar1=wneg_sb[:, 3:4], scalar2=None,
                    op0=Alu.add,
                )
                e3 = dpool.tile([COUT, S], bf16, tag="t")
                nc.vector.tensor_scalar(
                    out=e3[:].rearrange("p (h w) -> p h w", h=H, w=W),
                    in0=gvs[3], scalar1=-1.0, scalar2=wneg_sb[:, 3:4],
                    op0=Alu.mult, op1=Alu.subtract,
                )
                m3 = mpool.tile([COUT, S], bf16, tag="m")
                nc.vector.tensor_tensor(
                    out=m3[:], in0=d3[:], in1=e3[:], op=Alu.max
                )
                # taps 0-2: |G - w| on ScalarE (Abs activation, bias = -w)
                ts_ = []
                for n in range(3):
                    t = dpool.tile([COUT, S], bf16, tag="t")
                    nc.scalar.activation(
                        out=t[:].rearrange("p (h w) -> p h w", h=H, w=W),
                        in_=gvs[n], func=Act.Abs,
                        bias=wneg_sb[:, n : n + 1], scale=1.0,
                    )
                    ts_.append(t)
                # merge tree on VectorE; tap-3 branch is the tail
                m01 = mpool.tile([COUT, S], bf16, tag="m")
                nc.vector.tensor_tensor(
                    out=m01[:], in0=ts_[0][:], in1=ts_[1][:], op=Alu.max
                )
                m012 = mpool.tile([COUT, S], bf16, tag="m")
                nc.vector.tensor_tensor(
                    out=m012[:], in0=m01[:], in1=ts_[2][:], op=Alu.max
                )
                # final max + store in halves (overlap compute with store)
                outv = out_ext[b].rearrange("c h w -> c (h w)")
                for hh in range(2):
                    sl = slice(hh * (S // 2), (hh + 1) * (S // 2))
                    mf = opool.tile([COUT, S // 2], bf16, tag="o")
                    nc.vector.tensor_tensor(
                        out=mf[:], in0=m012[:, sl], in1=m3[:, sl], op=Alu.max
                    )
                    # out-store on the Scalar-engine HWDGE queue
                    nc.scalar.dma_start(out=outv[:, sl], in_=mf[:])
    nc.compile()
    return nc


def _host_inputs(x, weights, bias, conn_idx):
    """Per-core input maps (host-side prep: shard x, derive -w / gather
    row-indices from the tiny weight/index tensors)."""
    ci = np.asarray(conn_idx).astype(np.int64)          # [COUT, NCONN]
    c = ci // (KH * KW)
    rem = ci % (KH * KW)
    di = rem // KW
    dj = rem % KW
    # element offset into xpad_b [64, 66, 66]: c*4356 + di*66 + dj
    offs = (c * PLANE + di * PW + dj).astype(np.int32)          # [COUT, NCONN]
    gidx = np.zeros((COUT, BL * NCONN * 8), dtype=np.int32)
    for bb in range(BL):
        for n in range(NCONN):
            k = bb * NCONN + n
            gidx[:, k * 8] = offs[:, n]
    wneg = (-np.asarray(weights)).astype(np.float32)
    import ml_dtypes
    x = np.ascontiguousarray(np.asarray(x)).astype(ml_dtypes.bfloat16)
    in_maps = []
    for kcore in range(NCORES):
        in_maps.append(
            {
                "x": x[kcore * BL : (kcore + 1) * BL],
                "wneg": wneg,
                "gidx": gidx,
            }
        )
    return in_maps


def kernel(x, weights, bias, conn_idx):
    from concourse.bass_utils import run_bass_kernel_spmd

    if "nc" not in _CACHE:
        _CACHE["nc"] = _build_program()
    nc = _CACHE["nc"]
    in_maps = _host_inputs(x, weights, bias, conn_idx)
    res = run_bass_kernel_spmd(nc, in_maps, list(range(NCORES)))
    bias_f = np.asarray(bias, dtype=np.float32).reshape(1, COUT, 1, 1)
    outs = []
    for k in range(NCORES):
        a = np.stack(
            [
                np.asarray(res.results[k][f"out{b}"]).astype(np.float32)
                for b in range(BL)
            ]
        )
        outs.append(a + bias_f)    # exact f32 bias add on host
    return np.concatenate(outs, axis=0)


if __name__ == "__main__":
    nc = _build_program()
    print("program built OK")
